# revision 1
# baseline (speedup 1.0000x reference)
"""Trainium2 Bass kernel: causal self-attention with RL column mask.

Module: y = proj(softmax(mask(q @ k.T / sqrt(hd))) @ v) for B=4, T=2048,
C=1024, H=16 heads of 64. The mask is causal with every
(JOINED-1 + k*JOINED)-th key column disabled (JOINED=25).

Sharding over 8 NeuronCores: data-parallel on batch (4) x tensor-parallel on
heads (2 groups of 8). Each core computes q/k/v for its 8 heads from the full
x[b], runs attention with the full T locally (scores kept transposed
[t2, t1] so no on-chip transposes are needed; softmax denominator comes from
an appended ones-column in the AV matmul), and produces a partial output
projection. The host sums the two head-group partials per batch and adds the
bias terms (bp plus the folded v-bias term bv @ Wp_slice.T).
"""

from contextlib import ExitStack

import numpy as np

import concourse.mybir as mybir
import concourse.tile as tile
from concourse import bacc

F32 = mybir.dt.float32

B, T, C, H, HD = 4, 2048, 1024, 16, 64
JOINED = 25
NCORES = 8
MLOC = 512            # head channels per core (8 heads x 64)
NPAIR = MLOC // 128   # m-chunks of 128 = head pairs
NCC = C // 128        # contraction chunks for the qkv projections
NEG = -1.0e4          # additive mask value; exp(x <= -100) == 0.0 in fp32


def build_core_program(t=T, reps=1):
    """Bass program run identically on all 8 cores (inputs differ per core).

    reps > 1 re-emits the whole computation (for timing: the wall-clock
    difference between reps=r1 and reps=r2 programs isolates device time
    from the ~100ms axon dispatch overhead).
    """
    NJ = t // 128         # key (t2) chunks
    NQ = 4                # query (t1) quarters
    QW = t // NQ          # quarter width (<= 512 free dim per matmul)
    R = QW // 128         # key chunks per quarter
    NTB = 4               # x stream blocks in the qkv phase
    TBW = t // NTB
    assert QW <= 512 and t % 512 == 0

    nc = bacc.Bacc("TRN2", target_bir_lowering=False, debug=False)

    xT = nc.declare_dram_parameter("xT", [C, t], F32, isOutput=False)
    wqT = nc.declare_dram_parameter("wqT", [C, MLOC], F32, isOutput=False)
    wkT = nc.declare_dram_parameter("wkT", [C, MLOC], F32, isOutput=False)
    wvT = nc.declare_dram_parameter("wvT", [C, MLOC], F32, isOutput=False)
    wpT = nc.declare_dram_parameter("wpT", [MLOC, C], F32, isOutput=False)
    bqd = nc.declare_dram_parameter("bq", [MLOC], F32, isOutput=False)
    bkd = nc.declare_dram_parameter("bk", [MLOC], F32, isOutput=False)
    trid = nc.declare_dram_parameter("trimask", [128, 128], F32, isOutput=False)
    rld = nc.declare_dram_parameter("rlbias", [128, NJ], F32, isOutput=False)
    out_d = nc.declare_dram_parameter("out", [t, C], F32, isOutput=True)

    EXP = mybir.ActivationFunctionType.Exp

    with tile.TileContext(nc) as tc, ExitStack() as top:
        persist = top.enter_context(tc.tile_pool(name="persist", bufs=1))

        # persistent state: qT/kT [m, t] head-pair chunks, v [t2, 8*(64+1)]
        # per key chunk (column 64 of each 65-group holds the ones used for
        # the softmax denominator), plus small constants.
        qT = [persist.tile([128, t], F32, tag=f"qT{p}", name=f"qT{p}")
              for p in range(NPAIR)]
        kT = [persist.tile([128, t], F32, tag=f"kT{p}", name=f"kT{p}")
              for p in range(NPAIR)]
        vt = [persist.tile([128, 8 * 65], F32, tag=f"v{j}", name=f"v{j}")
              for j in range(NJ)]
        tri_t = persist.tile([128, 128], F32, tag="tri", name="tri_t")
        rl_t = persist.tile([128, NJ], F32, tag="rl", name="rl_t")
        bq_t = persist.tile([128, NPAIR], F32, tag="bq", name="bq_t")
        bk_t = persist.tile([128, NPAIR], F32, tag="bk", name="bk_t")
        ones_t = persist.tile([128, 64], F32, tag="ones", name="ones_t")

        nc.sync.dma_start(out=tri_t[:], in_=trid[:])
        nc.sync.dma_start(out=rl_t[:], in_=rld[:])
        nc.sync.dma_start(out=bq_t[:], in_=bqd.rearrange("(mc p) -> p mc", p=128))
        nc.sync.dma_start(out=bk_t[:], in_=bkd.rearrange("(mc p) -> p mc", p=128))
        nc.vector.memset(ones_t[:], 1.0)
        for j in range(NJ):
            nc.vector.memset(vt[j][:], 1.0)

        for rep in range(reps):
            # ---- phase 1: q/k/v projections, streaming x over t blocks ----
            with (
                tc.tile_pool(name="wpool", bufs=1) as wpool,
                tc.tile_pool(name="xpool", bufs=12) as xpool,
                tc.tile_pool(name="p1ps", bufs=1, space="PSUM") as p1ps,
            ):
                wq_t = [wpool.tile([128, MLOC], F32, tag=f"wq{c}", name=f"wq{c}")
                        for c in range(NCC)]
                wk_t = [wpool.tile([128, MLOC], F32, tag=f"wk{c}", name=f"wk{c}")
                        for c in range(NCC)]
                wv_t = [wpool.tile([128, MLOC], F32, tag=f"wv{c}", name=f"wv{c}")
                        for c in range(NCC)]
                for c in range(NCC):
                    cs = slice(128 * c, 128 * (c + 1))
                    nc.sync.dma_start(out=wq_t[c][:], in_=wqT[cs, :])
                    nc.sync.dma_start(out=wk_t[c][:], in_=wkT[cs, :])
                    nc.sync.dma_start(out=wv_t[c][:], in_=wvT[cs, :])

                for tb in range(NTB):
                    ts = slice(TBW * tb, TBW * (tb + 1))
                    xt = []
                    for c in range(NCC):
                        xc = xpool.tile([128, TBW], F32, tag="xt", name=f"x{tb}_{c}")
                        nc.sync.dma_start(out=xc[:], in_=xT[128 * c:128 * (c + 1), ts])
                        xt.append(xc)
                    # qT/kT chunks: out [m=128, t=TBW], contraction over c
                    for p in range(NPAIR):
                        ms = slice(128 * p, 128 * (p + 1))
                        for w_t, b_t, dst in ((wq_t, bq_t, qT), (wk_t, bk_t, kT)):
                            ps = p1ps.tile([128, TBW], F32, tag="qk", bufs=4,
                                           name=f"qk{tb}_{p}")
                            for c in range(NCC):
                                nc.tensor.matmul(ps[:], w_t[c][:, ms], xt[c][:],
                                                 start=(c == 0), stop=(c == NCC - 1))
                            nc.vector.tensor_scalar_add(dst[p][:, ts], ps[:],
                                                        b_t[:, p:p + 1])
                    # v chunks: out [t=128, m=512] (no bias: bv is folded on host)
                    for s4 in range(TBW // 128):
                        j = tb * (TBW // 128) + s4
                        ps = p1ps.tile([128, MLOC], F32, tag="vp", bufs=3,
                                       name=f"vp{j}")
                        for c in range(NCC):
                            nc.tensor.matmul(ps[:], xt[c][:, 128 * s4:128 * (s4 + 1)],
                                             wv_t[c][:], start=(c == 0),
                                             stop=(c == NCC - 1))
                        nc.vector.tensor_copy(
                            vt[j].rearrange("p (h d) -> p h d", d=65)[:, :, 0:64],
                            ps[:])

            # ---- phases 2+3 ----
            with (
                tc.tile_pool(name="ypool", bufs=1) as ypool,
                tc.tile_pool(name="wp2", bufs=1) as wp2,
            ):
                yT = [ypool.tile([128, t], F32, tag=f"yT{p}", name=f"yT{p}")
                      for p in range(NPAIR)]
                wp_t = [wp2.tile([128, C], F32, tag=f"wp{p}", name=f"wp{p}")
                        for p in range(NPAIR)]
                for p in range(NPAIR):
                    nc.sync.dma_start(out=wp_t[p][:], in_=wpT[128 * p:128 * (p + 1), :])

                # phase 2: attention per head pair p, per query quarter q.
                # Scores are computed transposed: S_T[t2, t1] in psum, one key
                # chunk at a time; exp (with 1/8 scale and the per-key RL mask
                # bias) evacuates them to sbuf; the AV matmul accumulates
                # [v | 1].T @ P_T into Y[65, QW] (row 64 = softmax denominator).
                with (
                    tc.tile_pool(name="ppool", bufs=1) as ppool,
                    tc.tile_pool(name="spool", bufs=1) as spool,
                    tc.tile_pool(name="p2ps", bufs=1, space="PSUM") as p2ps,
                ):
                    for p in range(NPAIR):
                        for q in range(NQ):
                            qs = slice(QW * q, QW * (q + 1))
                            y = [p2ps.tile([128, QW], F32, tag=f"y{h}", bufs=2,
                                           name=f"y{p}_{q}_{h}") for h in (0, 1)]
                            nj_q = R * (q + 1)
                            for j in range(nj_q):
                                o = max(0, 128 * j - QW * q)
                                w = QW - o
                                for h in (0, 1):
                                    hs = slice(64 * h, 64 * (h + 1))
                                    s_ps = p2ps.tile([128, w], F32, tag="st", bufs=3,
                                                     name=f"s{p}_{q}_{j}_{h}")
                                    nc.tensor.matmul(
                                        s_ps[:], kT[p][hs, 128 * j:128 * (j + 1)],
                                        qT[p][hs, QW * q + o:QW * (q + 1)],
                                        start=True, stop=True)
                                    pt = ppool.tile([128, w], F32, tag="pt", bufs=4,
                                                    name=f"pt{p}_{q}_{j}_{h}")
                                    nc.scalar.activation(pt[:], s_ps[:], EXP,
                                                         bias=rl_t[:, j:j + 1],
                                                         scale=0.125)
                                    if j >= R * q:
                                        # first 128 cols are the diagonal block
                                        nc.vector.tensor_mul(pt[:, 0:128],
                                                             pt[:, 0:128], tri_t[:])
                                    lh = 2 * p + h
                                    nc.tensor.matmul(
                                        y[h][0:65, o:o + w],
                                        vt[j][:, 65 * lh:65 * lh + 65], pt[:],
                                        start=(j == 0), stop=(j == nj_q - 1))
                            for h in (0, 1):
                                # custom DVE ops (reciprocal_approx_*) only work
                                # at base partition 0 on hardware, so broadcast
                                # the raw denominator down from partition 64
                                # first, then invert the broadcast tile.
                                den = spool.tile([128, QW], F32, tag="den", bufs=2,
                                                 name=f"den{p}_{q}_{h}")
                                nc.vector.tensor_copy(den[64:65, :], y[h][64:65, :])
                                bcp = p2ps.tile([64, QW], F32, tag="bc", bufs=1,
                                                name=f"bc{p}_{q}_{h}")
                                nc.tensor.matmul(bcp[0:64, :],
                                                 ones_t[64:65, 0:64],
                                                 den[64:65, :],
                                                 start=True, stop=True)
                                bcs = spool.tile([64, QW], F32, tag="bcs", bufs=2,
                                                 name=f"bcs{p}_{q}_{h}")
                                nc.vector.tensor_copy(bcs[:], bcp[0:64, :])
                                rbc = spool.tile([64, QW], F32, tag="rbc", bufs=2,
                                                 name=f"rbc{p}_{q}_{h}")
                                scr = spool.tile([64, QW], F32, tag="scr", bufs=2,
                                                 name=f"scr{p}_{q}_{h}")
                                nc.vector.reciprocal_approx_accurate(
                                    out=rbc[:], in_=bcs[:], scratch=scr[:])
                                if h == 0:
                                    nc.vector.tensor_mul(yT[p][0:64, qs],
                                                         y[h][0:64, :], rbc[:])
                                else:
                                    tmp = spool.tile([64, QW], F32, tag="tmp", bufs=2,
                                                     name=f"tmp{p}_{q}")
                                    nc.vector.tensor_mul(tmp[:], y[h][0:64, :], rbc[:])
                                    nc.sync.dma_start(out=yT[p][64:128, qs], in_=tmp[:])

                # phase 3: partial output projection out[t, i] (missing bias terms)
                with (
                    tc.tile_pool(name="p3ps", bufs=1, space="PSUM") as p3ps,
                    tc.tile_pool(name="opool", bufs=4) as opool,
                ):
                    for ih in range(C // 512):
                        for tb16 in range(t // 128):
                            op = p3ps.tile([128, 512], F32, tag="op", bufs=4,
                                           name=f"op{ih}_{tb16}")
                            for p in range(NPAIR):
                                nc.tensor.matmul(
                                    op[:], yT[p][:, 128 * tb16:128 * (tb16 + 1)],
                                    wp_t[p][:, 512 * ih:512 * (ih + 1)],
                                    start=(p == 0), stop=(p == NPAIR - 1))
                            ob = opool.tile([128, 512], F32, tag="ob",
                                            name=f"ob{ih}_{tb16}")
                            nc.vector.tensor_copy(ob[:], op[:])
                            nc.sync.dma_start(
                                out=out_d[128 * tb16:128 * (tb16 + 1),
                                          512 * ih:512 * (ih + 1)],
                                in_=ob[:])

    return nc


def make_masks(t=T):
    p = np.arange(128)[:, None]
    f = np.arange(128)[None, :]
    tri = (f >= p).astype(np.float32)
    nj = t // 128
    g = 128 * np.arange(nj)[None, :] + np.arange(128)[:, None]
    rl = np.where(g % JOINED == JOINED - 1, np.float32(NEG), np.float32(0.0))
    return tri, rl.astype(np.float32)


def make_in_maps(x, Wq, bq, Wk, bk, Wv, Wp):
    tri, rl = make_masks(T)
    in_maps = []
    for core in range(NCORES):
        b, hg = core // 2, core % 2
        sl = slice(hg * MLOC, (hg + 1) * MLOC)
        in_maps.append({
            "xT": np.ascontiguousarray(x[b].T),
            "wqT": np.ascontiguousarray(Wq[sl, :].T),
            "wkT": np.ascontiguousarray(Wk[sl, :].T),
            "wvT": np.ascontiguousarray(Wv[sl, :].T),
            "wpT": np.ascontiguousarray(Wp[:, sl].T),
            "bq": np.ascontiguousarray(bq[sl]),
            "bk": np.ascontiguousarray(bk[sl]),
            "trimask": tri,
            "rlbias": rl,
        })
    return in_maps


_NC_CACHE = None


def _get_nc():
    global _NC_CACHE
    if _NC_CACHE is None:
        nc = build_core_program(T)
        nc.finalize()
        _NC_CACHE = nc
    return _NC_CACHE


def kernel(x, Wq, bq, Wk, bk, Wv, bv, Wp, bp):
    from concourse.bass_utils import run_bass_kernel_spmd

    x = np.asarray(x, np.float32)
    Wq, bq = np.asarray(Wq, np.float32), np.asarray(bq, np.float32)
    Wk, bk = np.asarray(Wk, np.float32), np.asarray(bk, np.float32)
    Wv, bv = np.asarray(Wv, np.float32), np.asarray(bv, np.float32)
    Wp, bp = np.asarray(Wp, np.float32), np.asarray(bp, np.float32)

    nc = _get_nc()
    in_maps = make_in_maps(x, Wq, bq, Wk, bk, Wv, Wp)
    res = run_bass_kernel_spmd(nc, in_maps, list(range(NCORES)))

    # host gather: sum head-group partials, add bp and the folded bv term
    # (rows of the softmax sum to 1, so the v bias contributes bv @ Wp.T).
    const = bp + bv @ Wp.T
    out = np.empty((B, T, C), np.float32)
    for b in range(B):
        out[b] = res.results[2 * b]["out"] + res.results[2 * b + 1]["out"] + const
    return out


def build_core_program_v2(t=T, reps=1, mode="f32"):
    """Interleaved variant: qkv for t-quarter q is emitted, then attention
    for query quarter q (whose keys only span quarters 0..q, all already
    computed), then the projection for quarter q-1. This keeps TensorE fed
    while ScalarE chews on the exp evacuations instead of serial phases.

    mode selects the matmul operand dtype: "f32" (exact, 4 cycles/column on
    the PE), "f32r" (~13-bit mantissa, ~3 cycles), "bf16" (~8-bit mantissa,
    1 cycle). PSUM accumulation and the softmax-denominator path stay fp32
    in every mode.
    """
    D = {"f32": F32, "f32r": mybir.dt.float32r, "bf16": mybir.dt.bfloat16}[mode]
    NJ = t // 128
    NQ = 4
    QW = t // NQ
    R = QW // 128
    assert QW <= 512 and t % 512 == 0

    nc = bacc.Bacc("TRN2", target_bir_lowering=False, debug=False)

    xT = nc.declare_dram_parameter("xT", [C, t], F32, isOutput=False)
    wqT = nc.declare_dram_parameter("wqT", [C, MLOC], F32, isOutput=False)
    wkT = nc.declare_dram_parameter("wkT", [C, MLOC], F32, isOutput=False)
    wvT = nc.declare_dram_parameter("wvT", [C, MLOC], F32, isOutput=False)
    wpT = nc.declare_dram_parameter("wpT", [MLOC, C], F32, isOutput=False)
    bqd = nc.declare_dram_parameter("bq", [MLOC], F32, isOutput=False)
    bkd = nc.declare_dram_parameter("bk", [MLOC], F32, isOutput=False)
    trid = nc.declare_dram_parameter("trimask", [128, 128], F32, isOutput=False)
    rld = nc.declare_dram_parameter("rlbias", [128, NJ], F32, isOutput=False)
    out_d = nc.declare_dram_parameter("out", [t, C], F32, isOutput=True)

    EXP = mybir.ActivationFunctionType.Exp

    with tile.TileContext(nc) as tc, ExitStack() as top:
        persist = top.enter_context(tc.tile_pool(name="persist", bufs=1))
        wpool = top.enter_context(tc.tile_pool(name="wpool", bufs=1))
        xpool = top.enter_context(tc.tile_pool(name="xpool", bufs=8))
        qpool = top.enter_context(tc.tile_pool(name="qpool", bufs=2))
        ypool = top.enter_context(tc.tile_pool(name="ypool", bufs=2))
        ppool = top.enter_context(tc.tile_pool(name="ppool", bufs=4))
        spool = top.enter_context(tc.tile_pool(name="spool", bufs=1))
        opool = top.enter_context(tc.tile_pool(name="opool", bufs=2))
        psum = top.enter_context(tc.tile_pool(name="psum", bufs=1, space="PSUM"))

        kTq = [[persist.tile([128, QW], D, tag=f"kT{p}_{qq}",
                             name=f"kT{p}_{qq}") for qq in range(NQ)]
               for p in range(NPAIR)]
        vt = [persist.tile([128, 8 * 65], D, tag=f"v{j}", name=f"v{j}")
              for j in range(NJ)]
        tri_t = persist.tile([128, 128], F32, tag="tri", name="tri_t")
        rl_t = persist.tile([128, NJ], F32, tag="rl", name="rl_t")
        bq_t = persist.tile([128, NPAIR], F32, tag="bq", name="bq_t")
        bk_t = persist.tile([128, NPAIR], F32, tag="bk", name="bk_t")
        ones_t = persist.tile([128, 64], F32, tag="ones", name="ones_t")

        wq_t = [wpool.tile([128, MLOC], D, tag=f"wq{c}", name=f"wq{c}")
                for c in range(NCC)]
        wk_t = [wpool.tile([128, MLOC], D, tag=f"wk{c}", name=f"wk{c}")
                for c in range(NCC)]
        wv_t = [wpool.tile([128, MLOC], D, tag=f"wv{c}", name=f"wv{c}")
                for c in range(NCC)]
        wp_t = [wpool.tile([128, C], D, tag=f"wp{p}", name=f"wp{p}")
                for p in range(NPAIR)]

        nc.sync.dma_start(out=tri_t[:], in_=trid[:])
        nc.sync.dma_start(out=rl_t[:], in_=rld[:])
        nc.sync.dma_start(out=bq_t[:], in_=bqd.rearrange("(mc p) -> p mc", p=128))
        nc.sync.dma_start(out=bk_t[:], in_=bkd.rearrange("(mc p) -> p mc", p=128))
        nc.vector.memset(ones_t[:], 1.0)
        def load_rounded(dst, src_ap, stage_name):
            if D is F32:
                nc.sync.dma_start(out=dst[:], in_=src_ap)
            else:
                stg = xpool.tile(list(dst.shape), F32, tag="wstage",
                                 bufs=3, name=stage_name)
                nc.sync.dma_start(out=stg[:], in_=src_ap)
                nc.vector.tensor_copy(dst[:], stg[:])

        for c in range(NCC):
            cs = slice(128 * c, 128 * (c + 1))
            load_rounded(wq_t[c], wqT[cs, :], f"sq{c}")
            load_rounded(wk_t[c], wkT[cs, :], f"sk{c}")
            load_rounded(wv_t[c], wvT[cs, :], f"sv{c}")
        for p in range(NPAIR):
            load_rounded(wp_t[p], wpT[128 * p:128 * (p + 1), :], f"sp{p}")
        for j in range(NJ):
            nc.vector.memset(vt[j][:], 1.0)

        for rep in range(reps):
            qTq = {}   # p -> current quarter's q tile
            yTq = {}   # (p, q) -> y tile (consumed by proj of that quarter)

            def qkv_block(tb, rep=rep):
                ts = slice(QW * tb, QW * (tb + 1))
                xt = []
                for c in range(NCC):
                    xc = xpool.tile([128, QW], D, tag="xt",
                                    name=f"x{rep}_{tb}_{c}")
                    if D is F32:
                        nc.sync.dma_start(out=xc[:],
                                          in_=xT[128 * c:128 * (c + 1), ts])
                    else:
                        stg = xpool.tile([128, QW], F32, tag="xstage", bufs=3,
                                         name=f"xs{rep}_{tb}_{c}")
                        nc.sync.dma_start(out=stg[:],
                                          in_=xT[128 * c:128 * (c + 1), ts])
                        nc.vector.tensor_copy(xc[:], stg[:])
                    xt.append(xc)
                for p in range(NPAIR):
                    ms = slice(128 * p, 128 * (p + 1))
                    qt = qpool.tile([128, QW], D, tag=f"qT{p}",
                                    name=f"qT{rep}_{tb}_{p}")
                    qTq[p] = qt
                    for w_t, b_t, dst in ((wq_t, bq_t, qt), (wk_t, bk_t, kTq[p][tb])):
                        ps = psum.tile([128, QW], F32, tag="mm", bufs=3,
                                       name=f"qk{rep}_{tb}_{p}")
                        for c in range(NCC):
                            nc.tensor.matmul(ps[:], w_t[c][:, ms], xt[c][:],
                                             start=(c == 0), stop=(c == NCC - 1))
                        nc.vector.tensor_scalar_add(dst[:], ps[:], b_t[:, p:p + 1])
                for s4 in range(R):
                    j = tb * R + s4
                    ps = psum.tile([128, MLOC], F32, tag="mm", bufs=3,
                                   name=f"vp{rep}_{j}")
                    for c in range(NCC):
                        nc.tensor.matmul(ps[:], xt[c][:, 128 * s4:128 * (s4 + 1)],
                                         wv_t[c][:], start=(c == 0),
                                         stop=(c == NCC - 1))
                    nc.vector.tensor_copy(
                        vt[j].rearrange("p (h d) -> p h d", d=65)[:, :, 0:64],
                        ps[:])

            def att_block(q, rep=rep):
                qs = slice(QW * q, QW * (q + 1))
                for p in range(NPAIR):
                    y = [psum.tile([128, QW], F32, tag=f"y{h}", bufs=1,
                                   name=f"y{rep}_{p}_{q}_{h}") for h in (0, 1)]
                    nj_q = R * (q + 1)
                    for j in range(nj_q):
                        o = max(0, 128 * j - QW * q)
                        w = QW - o
                        for h in (0, 1):
                            hs = slice(64 * h, 64 * (h + 1))
                            s_ps = psum.tile([128, w], F32, tag="st", bufs=2,
                                             name=f"s{rep}_{p}_{q}_{j}_{h}")
                            nc.tensor.matmul(
                                s_ps[:], kTq[p][j // R][hs, 128 * (j % R):128 * (j % R) + 128],
                                qTq[p][hs, o:QW],
                                start=True, stop=True)
                            pt = ppool.tile([128, w], D, tag="pt", bufs=4,
                                            name=f"pt{rep}_{p}_{q}_{j}_{h}")
                            nc.scalar.activation(pt[:], s_ps[:], EXP,
                                                 bias=rl_t[:, j:j + 1],
                                                 scale=0.125)
                            if j >= R * q:
                                nc.vector.tensor_mul(pt[:, 0:128],
                                                     pt[:, 0:128], tri_t[:])
                            lh = 2 * p + h
                            nc.tensor.matmul(
                                y[h][0:65, o:o + w],
                                vt[j][:, 65 * lh:65 * lh + 65], pt[:],
                                start=(j == 0), stop=(j == nj_q - 1))
                    yt = ypool.tile([128, QW], D, tag=f"yT{p}",
                                    name=f"yT{rep}_{p}_{q}")
                    yTq[(p, q)] = yt
                    for h in (0, 1):
                        den = spool.tile([128, QW], F32, tag="den", bufs=1,
                                         name=f"den{rep}_{p}_{q}_{h}")
                        nc.vector.tensor_copy(den[64:65, :], y[h][64:65, :])
                        bcp = psum.tile([64, QW], F32, tag="bc", bufs=1,
                                        name=f"bc{rep}_{p}_{q}_{h}")
                        nc.tensor.matmul(bcp[0:64, :], ones_t[64:65, 0:64],
                                         den[64:65, :], start=True, stop=True)
                        bcs = spool.tile([64, QW], F32, tag="bcs", bufs=1,
                                         name=f"bcs{rep}_{p}_{q}_{h}")
                        nc.vector.tensor_copy(bcs[:], bcp[0:64, :])
                        rbc = spool.tile([64, QW], F32, tag="rbc", bufs=1,
                                         name=f"rbc{rep}_{p}_{q}_{h}")
                        scr = spool.tile([64, QW], F32, tag="scrtmp", bufs=1,
                                         name=f"scr{rep}_{p}_{q}_{h}")
                        nc.vector.reciprocal_approx_accurate(
                            out=rbc[:], in_=bcs[:], scratch=scr[:])
                        if h == 0:
                            nc.vector.tensor_mul(yt[0:64, :], y[h][0:64, :], rbc[:])
                        else:
                            tmp = spool.tile([64, QW], D, tag="tmp", bufs=1,
                                             name=f"tmp{rep}_{p}_{q}")
                            nc.vector.tensor_mul(tmp[:], y[h][0:64, :], rbc[:])
                            nc.sync.dma_start(out=yt[64:128, :], in_=tmp[:])

            def proj_block(q, rep=rep):
                for ih in range(C // 512):
                    for l4 in range(R):
                        tb16 = q * R + l4
                        op = psum.tile([128, 512], F32, tag="mm", bufs=3,
                                       name=f"op{rep}_{ih}_{tb16}")
                        for p in range(NPAIR):
                            nc.tensor.matmul(
                                op[:], yTq[(p, q)][:, 128 * l4:128 * (l4 + 1)],
                                wp_t[p][:, 512 * ih:512 * (ih + 1)],
                                start=(p == 0), stop=(p == NPAIR - 1))
                        ob = opool.tile([128, 512], F32, tag="ob",
                                        name=f"ob{rep}_{ih}_{tb16}")
                        nc.vector.tensor_copy(ob[:], op[:])
                        nc.sync.dma_start(
                            out=out_d[128 * tb16:128 * (tb16 + 1),
                                      512 * ih:512 * (ih + 1)],
                            in_=ob[:])

            qkv_block(0)
            att_block(0)
            qkv_block(1)
            att_block(1)
            proj_block(0)
            qkv_block(2)
            att_block(2)
            proj_block(1)
            qkv_block(3)
            att_block(3)
            proj_block(2)
            proj_block(3)

    return nc



# revision 15
# speedup vs baseline: 5.9911x; 5.9911x over previous
"""Trainium2 Bass kernel: causal self-attention with RL column mask.

Module: y = proj(softmax(mask(q @ k.T / sqrt(hd))) @ v) for B=4, T=2048,
C=1024, H=16 heads of 64. The mask is causal with every
(JOINED-1 + k*JOINED)-th key column disabled (JOINED=25).

Sharding over 8 NeuronCores: data-parallel on batch (4) x tensor-parallel on
heads (2 groups of 8). Each core computes q/k/v for its 8 heads from the full
x[b], runs attention with the full T locally (scores kept transposed
[t2, t1] so no on-chip transposes are needed; softmax denominator comes from
an appended ones-column in the AV matmul), and produces a partial output
projection. The host sums the two head-group partials per batch and adds the
bias terms (bp plus the folded v-bias term bv @ Wp_slice.T).
"""

from contextlib import ExitStack

import numpy as np

import concourse.mybir as mybir
import concourse.tile as tile
from concourse import bacc

F32 = mybir.dt.float32

B, T, C, H, HD = 4, 2048, 1024, 16, 64
JOINED = 25
NCORES = 8
MLOC = 512            # head channels per core (8 heads x 64)
NPAIR = MLOC // 128   # m-chunks of 128 = head pairs
NCC = C // 128        # contraction chunks for the qkv projections
NEG = -1.0e4          # additive mask value; exp(x <= -100) == 0.0 in fp32


def build_core_program(t=T, reps=1):
    """Bass program run identically on all 8 cores (inputs differ per core).

    reps > 1 re-emits the whole computation (for timing: the wall-clock
    difference between reps=r1 and reps=r2 programs isolates device time
    from the ~100ms axon dispatch overhead).
    """
    NJ = t // 128         # key (t2) chunks
    NQ = 4                # query (t1) quarters
    QW = t // NQ          # quarter width (<= 512 free dim per matmul)
    R = QW // 128         # key chunks per quarter
    NTB = 4               # x stream blocks in the qkv phase
    TBW = t // NTB
    assert QW <= 512 and t % 512 == 0

    nc = bacc.Bacc("TRN2", target_bir_lowering=False, debug=False)

    xT = nc.declare_dram_parameter("xT", [C, t], F32, isOutput=False)
    wqT = nc.declare_dram_parameter("wqT", [C, MLOC], F32, isOutput=False)
    wkT = nc.declare_dram_parameter("wkT", [C, MLOC], F32, isOutput=False)
    wvT = nc.declare_dram_parameter("wvT", [C, MLOC], F32, isOutput=False)
    wpT = nc.declare_dram_parameter("wpT", [MLOC, C], F32, isOutput=False)
    bqd = nc.declare_dram_parameter("bq", [MLOC], F32, isOutput=False)
    bkd = nc.declare_dram_parameter("bk", [MLOC], F32, isOutput=False)
    trid = nc.declare_dram_parameter("trimask", [128, 128], F32, isOutput=False)
    rld = nc.declare_dram_parameter("rlbias", [128, NJ], F32, isOutput=False)
    out_d = nc.declare_dram_parameter("out", [t, C], F32, isOutput=True)

    EXP = mybir.ActivationFunctionType.Exp

    with tile.TileContext(nc) as tc, ExitStack() as top:
        persist = top.enter_context(tc.tile_pool(name="persist", bufs=1))

        # persistent state: qT/kT [m, t] head-pair chunks, v [t2, 8*(64+1)]
        # per key chunk (column 64 of each 65-group holds the ones used for
        # the softmax denominator), plus small constants.
        qT = [persist.tile([128, t], F32, tag=f"qT{p}", name=f"qT{p}")
              for p in range(NPAIR)]
        kT = [persist.tile([128, t], F32, tag=f"kT{p}", name=f"kT{p}")
              for p in range(NPAIR)]
        vt = [persist.tile([128, 8 * 65], F32, tag=f"v{j}", name=f"v{j}")
              for j in range(NJ)]
        tri_t = persist.tile([128, 128], F32, tag="tri", name="tri_t")
        rl_t = persist.tile([128, NJ], F32, tag="rl", name="rl_t")
        bq_t = persist.tile([128, NPAIR], F32, tag="bq", name="bq_t")
        bk_t = persist.tile([128, NPAIR], F32, tag="bk", name="bk_t")
        ones_t = persist.tile([128, 64], F32, tag="ones", name="ones_t")

        nc.sync.dma_start(out=tri_t[:], in_=trid[:])
        nc.sync.dma_start(out=rl_t[:], in_=rld[:])
        nc.sync.dma_start(out=bq_t[:], in_=bqd.rearrange("(mc p) -> p mc", p=128))
        nc.sync.dma_start(out=bk_t[:], in_=bkd.rearrange("(mc p) -> p mc", p=128))
        nc.vector.memset(ones_t[:], 1.0)
        for j in range(NJ):
            nc.vector.memset(vt[j][:], 1.0)

        for rep in range(reps):
            # ---- phase 1: q/k/v projections, streaming x over t blocks ----
            with (
                tc.tile_pool(name="wpool", bufs=1) as wpool,
                tc.tile_pool(name="xpool", bufs=12) as xpool,
                tc.tile_pool(name="p1ps", bufs=1, space="PSUM") as p1ps,
            ):
                wq_t = [wpool.tile([128, MLOC], F32, tag=f"wq{c}", name=f"wq{c}")
                        for c in range(NCC)]
                wk_t = [wpool.tile([128, MLOC], F32, tag=f"wk{c}", name=f"wk{c}")
                        for c in range(NCC)]
                wv_t = [wpool.tile([128, MLOC], F32, tag=f"wv{c}", name=f"wv{c}")
                        for c in range(NCC)]
                for c in range(NCC):
                    cs = slice(128 * c, 128 * (c + 1))
                    nc.sync.dma_start(out=wq_t[c][:], in_=wqT[cs, :])
                    nc.sync.dma_start(out=wk_t[c][:], in_=wkT[cs, :])
                    nc.sync.dma_start(out=wv_t[c][:], in_=wvT[cs, :])

                for tb in range(NTB):
                    ts = slice(TBW * tb, TBW * (tb + 1))
                    xt = []
                    for c in range(NCC):
                        xc = xpool.tile([128, TBW], F32, tag="xt", name=f"x{tb}_{c}")
                        nc.sync.dma_start(out=xc[:], in_=xT[128 * c:128 * (c + 1), ts])
                        xt.append(xc)
                    # qT/kT chunks: out [m=128, t=TBW], contraction over c
                    for p in range(NPAIR):
                        ms = slice(128 * p, 128 * (p + 1))
                        for w_t, b_t, dst in ((wq_t, bq_t, qT), (wk_t, bk_t, kT)):
                            ps = p1ps.tile([128, TBW], F32, tag="qk", bufs=4,
                                           name=f"qk{tb}_{p}")
                            for c in range(NCC):
                                nc.tensor.matmul(ps[:], w_t[c][:, ms], xt[c][:],
                                                 start=(c == 0), stop=(c == NCC - 1))
                            nc.vector.tensor_scalar_add(dst[p][:, ts], ps[:],
                                                        b_t[:, p:p + 1])
                    # v chunks: out [t=128, m=512] (no bias: bv is folded on host)
                    for s4 in range(TBW // 128):
                        j = tb * (TBW // 128) + s4
                        ps = p1ps.tile([128, MLOC], F32, tag="vp", bufs=3,
                                       name=f"vp{j}")
                        for c in range(NCC):
                            nc.tensor.matmul(ps[:], xt[c][:, 128 * s4:128 * (s4 + 1)],
                                             wv_t[c][:], start=(c == 0),
                                             stop=(c == NCC - 1))
                        nc.vector.tensor_copy(
                            vt[j].rearrange("p (h d) -> p h d", d=65)[:, :, 0:64],
                            ps[:])

            # ---- phases 2+3 ----
            with (
                tc.tile_pool(name="ypool", bufs=1) as ypool,
                tc.tile_pool(name="wp2", bufs=1) as wp2,
            ):
                yT = [ypool.tile([128, t], F32, tag=f"yT{p}", name=f"yT{p}")
                      for p in range(NPAIR)]
                wp_t = [wp2.tile([128, C], F32, tag=f"wp{p}", name=f"wp{p}")
                        for p in range(NPAIR)]
                for p in range(NPAIR):
                    nc.sync.dma_start(out=wp_t[p][:], in_=wpT[128 * p:128 * (p + 1), :])

                # phase 2: attention per head pair p, per query quarter q.
                # Scores are computed transposed: S_T[t2, t1] in psum, one key
                # chunk at a time; exp (with 1/8 scale and the per-key RL mask
                # bias) evacuates them to sbuf; the AV matmul accumulates
                # [v | 1].T @ P_T into Y[65, QW] (row 64 = softmax denominator).
                with (
                    tc.tile_pool(name="ppool", bufs=1) as ppool,
                    tc.tile_pool(name="spool", bufs=1) as spool,
                    tc.tile_pool(name="p2ps", bufs=1, space="PSUM") as p2ps,
                ):
                    for p in range(NPAIR):
                        for q in range(NQ):
                            qs = slice(QW * q, QW * (q + 1))
                            y = [p2ps.tile([128, QW], F32, tag=f"y{h}", bufs=2,
                                           name=f"y{p}_{q}_{h}") for h in (0, 1)]
                            nj_q = R * (q + 1)
                            for j in range(nj_q):
                                o = max(0, 128 * j - QW * q)
                                w = QW - o
                                for h in (0, 1):
                                    hs = slice(64 * h, 64 * (h + 1))
                                    s_ps = p2ps.tile([128, w], F32, tag="st", bufs=3,
                                                     name=f"s{p}_{q}_{j}_{h}")
                                    nc.tensor.matmul(
                                        s_ps[:], kT[p][hs, 128 * j:128 * (j + 1)],
                                        qT[p][hs, QW * q + o:QW * (q + 1)],
                                        start=True, stop=True)
                                    pt = ppool.tile([128, w], F32, tag="pt", bufs=4,
                                                    name=f"pt{p}_{q}_{j}_{h}")
                                    nc.scalar.activation(pt[:], s_ps[:], EXP,
                                                         bias=rl_t[:, j:j + 1],
                                                         scale=0.125)
                                    if j >= R * q:
                                        # first 128 cols are the diagonal block
                                        nc.vector.tensor_mul(pt[:, 0:128],
                                                             pt[:, 0:128], tri_t[:])
                                    lh = 2 * p + h
                                    nc.tensor.matmul(
                                        y[h][0:65, o:o + w],
                                        vt[j][:, 65 * lh:65 * lh + 65], pt[:],
                                        start=(j == 0), stop=(j == nj_q - 1))
                            for h in (0, 1):
                                # custom DVE ops (reciprocal_approx_*) only work
                                # at base partition 0 on hardware, so broadcast
                                # the raw denominator down from partition 64
                                # first, then invert the broadcast tile.
                                den = spool.tile([128, QW], F32, tag="den", bufs=2,
                                                 name=f"den{p}_{q}_{h}")
                                nc.vector.tensor_copy(den[64:65, :], y[h][64:65, :])
                                bcp = p2ps.tile([64, QW], F32, tag="bc", bufs=1,
                                                name=f"bc{p}_{q}_{h}")
                                nc.tensor.matmul(bcp[0:64, :],
                                                 ones_t[64:65, 0:64],
                                                 den[64:65, :],
                                                 start=True, stop=True)
                                bcs = spool.tile([64, QW], F32, tag="bcs", bufs=2,
                                                 name=f"bcs{p}_{q}_{h}")
                                nc.vector.tensor_copy(bcs[:], bcp[0:64, :])
                                rbc = spool.tile([64, QW], F32, tag="rbc", bufs=2,
                                                 name=f"rbc{p}_{q}_{h}")
                                scr = spool.tile([64, QW], F32, tag="scr", bufs=2,
                                                 name=f"scr{p}_{q}_{h}")
                                nc.vector.reciprocal_approx_accurate(
                                    out=rbc[:], in_=bcs[:], scratch=scr[:])
                                if h == 0:
                                    nc.vector.tensor_mul(yT[p][0:64, qs],
                                                         y[h][0:64, :], rbc[:])
                                else:
                                    tmp = spool.tile([64, QW], F32, tag="tmp", bufs=2,
                                                     name=f"tmp{p}_{q}")
                                    nc.vector.tensor_mul(tmp[:], y[h][0:64, :], rbc[:])
                                    nc.sync.dma_start(out=yT[p][64:128, qs], in_=tmp[:])

                # phase 3: partial output projection out[t, i] (missing bias terms)
                with (
                    tc.tile_pool(name="p3ps", bufs=1, space="PSUM") as p3ps,
                    tc.tile_pool(name="opool", bufs=4) as opool,
                ):
                    for ih in range(C // 512):
                        for tb16 in range(t // 128):
                            op = p3ps.tile([128, 512], F32, tag="op", bufs=4,
                                           name=f"op{ih}_{tb16}")
                            for p in range(NPAIR):
                                nc.tensor.matmul(
                                    op[:], yT[p][:, 128 * tb16:128 * (tb16 + 1)],
                                    wp_t[p][:, 512 * ih:512 * (ih + 1)],
                                    start=(p == 0), stop=(p == NPAIR - 1))
                            ob = opool.tile([128, 512], F32, tag="ob",
                                            name=f"ob{ih}_{tb16}")
                            nc.vector.tensor_copy(ob[:], op[:])
                            nc.sync.dma_start(
                                out=out_d[128 * tb16:128 * (tb16 + 1),
                                          512 * ih:512 * (ih + 1)],
                                in_=ob[:])

    return nc


def make_masks(t=T):
    p = np.arange(128)[:, None]
    f = np.arange(128)[None, :]
    tri = (f >= p).astype(np.float32)
    nj = t // 128
    g = 128 * np.arange(nj)[None, :] + np.arange(128)[:, None]
    rl = np.where(g % JOINED == JOINED - 1, np.float32(NEG), np.float32(0.0))
    return tri, rl.astype(np.float32)


def make_in_maps(x, Wq, bq, Wk, bk, Wv, Wp):
    tri, rl = make_masks(T)
    in_maps = []
    for core in range(NCORES):
        b, hg = core // 2, core % 2
        sl = slice(hg * MLOC, (hg + 1) * MLOC)
        in_maps.append({
            "xT": np.ascontiguousarray(x[b].T),
            "wqT": np.ascontiguousarray(Wq[sl, :].T),
            "wkT": np.ascontiguousarray(Wk[sl, :].T),
            "wvT": np.ascontiguousarray(Wv[sl, :].T),
            "wpT": np.ascontiguousarray(Wp[:, sl].T),
            "bq": np.ascontiguousarray(bq[sl]),
            "bk": np.ascontiguousarray(bk[sl]),
            "trimask": tri,
            "rlbias": rl,
            "ones": np.ones((128, 8 * 65), np.float32),
        })
    return in_maps


def build_core_program_v3(t=T, reps=1):
    """f32r + interleaved phases + cheap softmax normalization.

    All matmuls take float32r-bitcast operands (fp32 data, ~13-bit mantissa,
    1 PE cycle/row at free size >= 256 vs 4 for fp32). Phases are interleaved
    per t-quarter so TensorE keeps running while ScalarE handles the exp
    evacuations. Per (pair, quarter) the two heads' AV outputs land at psum
    partitions [0:65) (v|1 layout, denominator row last) and [63:128) (1|v
    layout, denominator row first), so one broadcast psum tile + a single
    reciprocal serves both heads and the normalizing muls are lane-aligned
    (no sbuf-shift DMA).
    """
    F32R = mybir.dt.float32r
    NJ = t // 128
    NQ = 4
    QW = t // NQ
    R = QW // 128
    assert QW <= 512 and t % 512 == 0

    nc = bacc.Bacc("TRN2", target_bir_lowering=False, debug=False)

    xT = nc.declare_dram_parameter("xT", [C, t], F32R, isOutput=False)
    wqT = nc.declare_dram_parameter("wqT", [C, MLOC], F32R, isOutput=False)
    wkT = nc.declare_dram_parameter("wkT", [C, MLOC], F32R, isOutput=False)
    wvT = nc.declare_dram_parameter("wvT", [C, MLOC], F32R, isOutput=False)
    wpT = nc.declare_dram_parameter("wpT", [MLOC, C], F32R, isOutput=False)
    bqd = nc.declare_dram_parameter("bq", [MLOC], F32, isOutput=False)
    bkd = nc.declare_dram_parameter("bk", [MLOC], F32, isOutput=False)
    trid = nc.declare_dram_parameter("trimask", [128, 128], F32R, isOutput=False)
    rld = nc.declare_dram_parameter("rlbias", [128, NJ], F32, isOutput=False)
    onesd = nc.declare_dram_parameter("ones", [128, 8 * 65], F32R,
                                      isOutput=False)
    out_d = nc.declare_dram_parameter("out", [t, C], F32, isOutput=True)

    EXP = mybir.ActivationFunctionType.Exp

    def rr(ap):
        # tiles feeding matmuls are declared F32R already; identity kept so
        # operand sites read uniformly
        return ap

    with tile.TileContext(nc) as tc, ExitStack() as top:
        persist = top.enter_context(tc.tile_pool(name="persist", bufs=1))
        wpool = top.enter_context(tc.tile_pool(name="wpool", bufs=1))
        xpool = top.enter_context(tc.tile_pool(name="xpool", bufs=8))
        qpool = top.enter_context(tc.tile_pool(name="qpool", bufs=2))
        ypool = top.enter_context(tc.tile_pool(name="ypool", bufs=2))
        ppool = top.enter_context(tc.tile_pool(name="ppool", bufs=3))
        spool = top.enter_context(tc.tile_pool(name="spool", bufs=2))
        opool = top.enter_context(tc.tile_pool(name="opool", bufs=2))
        psum = top.enter_context(tc.tile_pool(name="psum", bufs=1, space="PSUM"))

        kTq = [[persist.tile([128, QW], F32R, tag=f"kT{p}_{qq}",
                             name=f"kT{p}_{qq}") for qq in range(NQ)]
               for p in range(NPAIR)]
        # v tiles: per t2 chunk j, 8 head groups of 65 cols: [v(64) | 1].
        # Each head's AV output lands at psum base 0 (channels 0..63,
        # denominator at row 64).
        vt = [persist.tile([128, 8 * 65], F32R, tag=f"v{j}", name=f"v{j}")
              for j in range(NJ)]
        tri_t = persist.tile([128, 128], F32R, tag="tri", name="tri_t")
        rl_t = persist.tile([128, NJ], F32, tag="rl", name="rl_t")
        bq_t = persist.tile([128, NPAIR], F32, tag="bq", name="bq_t")
        bk_t = persist.tile([128, NPAIR], F32, tag="bk", name="bk_t")
        ones_t = persist.tile([128, 64], F32R, tag="ones", name="ones_t")

        wq_t = [wpool.tile([128, MLOC], F32R, tag=f"wq{c}", name=f"wq{c}")
                for c in range(NCC)]
        wk_t = [wpool.tile([128, MLOC], F32R, tag=f"wk{c}", name=f"wk{c}")
                for c in range(NCC)]
        wv_t = [wpool.tile([128, MLOC], F32R, tag=f"wv{c}", name=f"wv{c}")
                for c in range(NCC)]
        wp_t = [wpool.tile([128, C], F32R, tag=f"wp{p}", name=f"wp{p}")
                for p in range(NPAIR)]

        nc.sync.dma_start(out=tri_t[:], in_=trid[:])
        nc.sync.dma_start(out=rl_t[:], in_=rld[:])
        nc.sync.dma_start(out=bq_t[:], in_=bqd.rearrange("(mc p) -> p mc", p=128))
        nc.sync.dma_start(out=bk_t[:], in_=bkd.rearrange("(mc p) -> p mc", p=128))
        nc.sync.dma_start(out=ones_t[:], in_=onesd[:, 0:64])
        for c in range(NCC):
            cs = slice(128 * c, 128 * (c + 1))
            nc.sync.dma_start(out=wq_t[c][:], in_=wqT[cs, :])
            nc.sync.dma_start(out=wk_t[c][:], in_=wkT[cs, :])
            nc.sync.dma_start(out=wv_t[c][:], in_=wvT[cs, :])
        for p in range(NPAIR):
            nc.sync.dma_start(out=wp_t[p][:], in_=wpT[128 * p:128 * (p + 1), :])
        for j in range(NJ):
            nc.sync.dma_start(out=vt[j][:], in_=onesd[:])

        for rep in range(reps):
            qTq = {}   # p -> current quarter's qT tile
            yTq = {}   # (p, q) -> normalized head outputs [m=128, QW]

            def qkv_block(tb, rep=rep):
                ts = slice(QW * tb, QW * (tb + 1))
                xt = []
                for c in range(NCC):
                    xc = xpool.tile([128, QW], F32R, tag="xt",
                                    name=f"x{rep}_{tb}_{c}")
                    nc.sync.dma_start(out=xc[:], in_=xT[128 * c:128 * (c + 1), ts])
                    xt.append(xc)
                for p in range(NPAIR):
                    ms = slice(128 * p, 128 * (p + 1))
                    qt = qpool.tile([128, QW], F32R, tag=f"qT{p}",
                                    name=f"qT{rep}_{tb}_{p}")
                    qTq[p] = qt
                    for w_t, b_t, dst in ((wq_t, bq_t, qt),
                                          (wk_t, bk_t, kTq[p][tb])):
                        ps = psum.tile([128, QW], F32, tag="mm", bufs=2,
                                       name=f"qk{rep}_{tb}_{p}")
                        for c in range(NCC):
                            nc.tensor.matmul(ps[:], rr(w_t[c][:, ms]),
                                             rr(xt[c][:]),
                                             start=(c == 0), stop=(c == NCC - 1))
                        nc.vector.tensor_scalar_add(dst[:], ps[:], b_t[:, p:p + 1])
                for s4 in range(R):
                    j = tb * R + s4
                    ps = psum.tile([128, MLOC], F32, tag="mm", bufs=2,
                                   name=f"vp{rep}_{j}")
                    for c in range(NCC):
                        nc.tensor.matmul(ps[:], rr(xt[c][:, 128 * s4:128 * (s4 + 1)]),
                                         rr(wv_t[c][:]), start=(c == 0),
                                         stop=(c == NCC - 1))
                    nc.vector.tensor_copy(
                        vt[j].rearrange("p (h d) -> p h d", d=65)[:, :, 0:64],
                        ps[:])

            def att_block(q, rep=rep):
                for p in range(NPAIR):
                    y0 = psum.tile([128, QW], F32, tag="y0", bufs=1,
                                   name=f"y0_{rep}_{p}_{q}")
                    y1 = psum.tile([128, QW], F32, tag="y1", bufs=1,
                                   name=f"y1_{rep}_{p}_{q}")
                    nj_q = R * (q + 1)
                    for j in range(nj_q):
                        o = max(0, 128 * j - QW * q)
                        w = QW - o
                        for h in (0, 1):
                            hs = slice(64 * h, 64 * (h + 1))
                            s_ps = psum.tile([128, w], F32, tag="st", bufs=2,
                                             name=f"s{rep}_{p}_{q}_{j}_{h}")
                            nc.tensor.matmul(
                                s_ps[:],
                                rr(kTq[p][j // R][hs, 128 * (j % R):
                                                  128 * (j % R) + 128]),
                                rr(qTq[p][hs, o:QW]),
                                start=True, stop=True)
                            pt = ppool.tile([128, w], F32R, tag="pt", bufs=4,
                                            name=f"pt{rep}_{p}_{q}_{j}_{h}")
                            nc.scalar.activation(pt[:], s_ps[:], EXP,
                                                 bias=rl_t[:, j:j + 1],
                                                 scale=0.125)
                            if j >= R * q:
                                nc.vector.tensor_mul(pt[:, 0:128],
                                                     pt[:, 0:128], tri_t[:])
                            lh = 2 * p + h
                            yh = y0 if h == 0 else y1
                            nc.tensor.matmul(
                                yh[0:65, o:o + w],
                                rr(vt[j][:, 65 * lh:65 * lh + 65]),
                                rr(pt[:]),
                                start=(j == 0), stop=(j == nj_q - 1))
                    # normalization: both denominators sit at psum partition
                    # 64 of their tiles. Stage into adjacent column halves of
                    # one sbuf row, broadcast each across partitions 0..63 of
                    # one two-bank psum tile via K=1 matmuls, invert both in
                    # a single reciprocal, multiply lane-aligned. h1's result
                    # reaches yT partitions 64:128 via a small sbuf DMA.
                    den = spool.tile([128, 2 * QW], F32R, tag="den", bufs=1,
                                     name=f"den{rep}_{p}_{q}")
                    nc.vector.tensor_copy(den[64:65, 0:QW], y0[64:65, :])
                    nc.vector.tensor_copy(den[64:65, QW:2 * QW], y1[64:65, :])
                    bc = psum.tile([64, 2 * QW], F32, tag="bc", bufs=1,
                                   name=f"bc{rep}_{p}_{q}")
                    nc.tensor.matmul(bc[0:64, 0:QW], rr(ones_t[64:65, 0:64]),
                                     rr(den[64:65, 0:QW]), start=True,
                                     stop=True)
                    nc.tensor.matmul(bc[0:64, QW:2 * QW],
                                     rr(ones_t[64:65, 0:64]),
                                     rr(den[64:65, QW:2 * QW]), start=True,
                                     stop=True)
                    rbc = spool.tile([64, 2 * QW], F32, tag="rbc", bufs=1,
                                     name=f"rbc{rep}_{p}_{q}")
                    scr = spool.tile([64, 2 * QW], F32, tag="scr", bufs=1,
                                     name=f"scr{rep}_{p}_{q}")
                    nc.vector.reciprocal_approx_accurate(
                        out=rbc[:], in_=bc[:], scratch=scr[:])
                    yt = ypool.tile([128, QW], F32R, tag=f"yT{p}",
                                    name=f"yT{rep}_{p}_{q}")
                    yTq[(p, q)] = yt
                    nc.vector.tensor_mul(yt[0:64, :], y0[0:64, :],
                                         rbc[:, 0:QW])
                    tmp = spool.tile([64, QW], F32R, tag="tmp", bufs=2,
                                     name=f"tmp{rep}_{p}_{q}")
                    nc.vector.tensor_mul(tmp[:], y1[0:64, :], rbc[:, QW:2 * QW])
                    nc.sync.dma_start(out=yt[64:128, :], in_=tmp[:])

            def proj_block(q, rep=rep):
                for ih in range(C // 512):
                    for l4 in range(R):
                        tb16 = q * R + l4
                        op = psum.tile([128, 512], F32, tag="mm", bufs=2,
                                       name=f"op{rep}_{ih}_{tb16}")
                        for p in range(NPAIR):
                            nc.tensor.matmul(
                                op[:], rr(yTq[(p, q)][:, 128 * l4:128 * (l4 + 1)]),
                                rr(wp_t[p][:, 512 * ih:512 * (ih + 1)]),
                                start=(p == 0), stop=(p == NPAIR - 1))
                        ob = opool.tile([128, 512], F32, tag="ob",
                                        name=f"ob{rep}_{ih}_{tb16}")
                        nc.vector.tensor_copy(ob[:], op[:])
                        nc.sync.dma_start(
                            out=out_d[128 * tb16:128 * (tb16 + 1),
                                      512 * ih:512 * (ih + 1)],
                            in_=ob[:])

            qkv_block(0)
            att_block(0)
            qkv_block(1)
            att_block(1)
            proj_block(0)
            qkv_block(2)
            att_block(2)
            proj_block(1)
            qkv_block(3)
            att_block(3)
            proj_block(2)
            proj_block(3)

    return nc


_NC_CACHE = None


def _get_nc():
    global _NC_CACHE
    if _NC_CACHE is None:
        nc = build_core_program_v3(T)
        nc.finalize()
        _NC_CACHE = nc
    return _NC_CACHE


def kernel(x, Wq, bq, Wk, bk, Wv, bv, Wp, bp):
    from concourse.bass_utils import run_bass_kernel_spmd

    x = np.asarray(x, np.float32)
    Wq, bq = np.asarray(Wq, np.float32), np.asarray(bq, np.float32)
    Wk, bk = np.asarray(Wk, np.float32), np.asarray(bk, np.float32)
    Wv, bv = np.asarray(Wv, np.float32), np.asarray(bv, np.float32)
    Wp, bp = np.asarray(Wp, np.float32), np.asarray(bp, np.float32)

    nc = _get_nc()
    in_maps = make_in_maps(x, Wq, bq, Wk, bk, Wv, Wp)
    res = run_bass_kernel_spmd(nc, in_maps, list(range(NCORES)))

    # host gather: sum head-group partials, add bp and the folded bv term
    # (rows of the softmax sum to 1, so the v bias contributes bv @ Wp.T).
    const = bp + bv @ Wp.T
    out = np.empty((B, T, C), np.float32)
    for b in range(B):
        out[b] = res.results[2 * b]["out"] + res.results[2 * b + 1]["out"] + const
    return out


def build_core_program_v2(t=T, reps=1, mode="f32"):
    """Interleaved variant: qkv for t-quarter q is emitted, then attention
    for query quarter q (whose keys only span quarters 0..q, all already
    computed), then the projection for quarter q-1. This keeps TensorE fed
    while ScalarE chews on the exp evacuations instead of serial phases.

    mode selects the matmul operand dtype: "f32" (exact, 4 cycles/column on
    the PE), "f32r" (~13-bit mantissa, ~3 cycles), "bf16" (~8-bit mantissa,
    1 cycle). PSUM accumulation and the softmax-denominator path stay fp32
    in every mode.
    """
    D = {"f32": F32, "f32r": mybir.dt.float32r, "bf16": mybir.dt.bfloat16}[mode]
    NJ = t // 128
    NQ = 4
    QW = t // NQ
    R = QW // 128
    assert QW <= 512 and t % 512 == 0

    nc = bacc.Bacc("TRN2", target_bir_lowering=False, debug=False)

    xT = nc.declare_dram_parameter("xT", [C, t], F32, isOutput=False)
    wqT = nc.declare_dram_parameter("wqT", [C, MLOC], F32, isOutput=False)
    wkT = nc.declare_dram_parameter("wkT", [C, MLOC], F32, isOutput=False)
    wvT = nc.declare_dram_parameter("wvT", [C, MLOC], F32, isOutput=False)
    wpT = nc.declare_dram_parameter("wpT", [MLOC, C], F32, isOutput=False)
    bqd = nc.declare_dram_parameter("bq", [MLOC], F32, isOutput=False)
    bkd = nc.declare_dram_parameter("bk", [MLOC], F32, isOutput=False)
    trid = nc.declare_dram_parameter("trimask", [128, 128], F32, isOutput=False)
    rld = nc.declare_dram_parameter("rlbias", [128, NJ], F32, isOutput=False)
    out_d = nc.declare_dram_parameter("out", [t, C], F32, isOutput=True)

    EXP = mybir.ActivationFunctionType.Exp

    with tile.TileContext(nc) as tc, ExitStack() as top:
        persist = top.enter_context(tc.tile_pool(name="persist", bufs=1))
        wpool = top.enter_context(tc.tile_pool(name="wpool", bufs=1))
        xpool = top.enter_context(tc.tile_pool(name="xpool", bufs=8))
        qpool = top.enter_context(tc.tile_pool(name="qpool", bufs=2))
        ypool = top.enter_context(tc.tile_pool(name="ypool", bufs=2))
        ppool = top.enter_context(tc.tile_pool(name="ppool", bufs=4))
        spool = top.enter_context(tc.tile_pool(name="spool", bufs=1))
        opool = top.enter_context(tc.tile_pool(name="opool", bufs=2))
        psum = top.enter_context(tc.tile_pool(name="psum", bufs=1, space="PSUM"))

        kTq = [[persist.tile([128, QW], D, tag=f"kT{p}_{qq}",
                             name=f"kT{p}_{qq}") for qq in range(NQ)]
               for p in range(NPAIR)]
        vt = [persist.tile([128, 8 * 65], D, tag=f"v{j}", name=f"v{j}")
              for j in range(NJ)]
        tri_t = persist.tile([128, 128], F32, tag="tri", name="tri_t")
        rl_t = persist.tile([128, NJ], F32, tag="rl", name="rl_t")
        bq_t = persist.tile([128, NPAIR], F32, tag="bq", name="bq_t")
        bk_t = persist.tile([128, NPAIR], F32, tag="bk", name="bk_t")
        ones_t = persist.tile([128, 64], F32, tag="ones", name="ones_t")

        wq_t = [wpool.tile([128, MLOC], D, tag=f"wq{c}", name=f"wq{c}")
                for c in range(NCC)]
        wk_t = [wpool.tile([128, MLOC], D, tag=f"wk{c}", name=f"wk{c}")
                for c in range(NCC)]
        wv_t = [wpool.tile([128, MLOC], D, tag=f"wv{c}", name=f"wv{c}")
                for c in range(NCC)]
        wp_t = [wpool.tile([128, C], D, tag=f"wp{p}", name=f"wp{p}")
                for p in range(NPAIR)]

        nc.sync.dma_start(out=tri_t[:], in_=trid[:])
        nc.sync.dma_start(out=rl_t[:], in_=rld[:])
        nc.sync.dma_start(out=bq_t[:], in_=bqd.rearrange("(mc p) -> p mc", p=128))
        nc.sync.dma_start(out=bk_t[:], in_=bkd.rearrange("(mc p) -> p mc", p=128))
        nc.vector.memset(ones_t[:], 1.0)
        def load_rounded(dst, src_ap, stage_name):
            if D is F32:
                nc.sync.dma_start(out=dst[:], in_=src_ap)
            else:
                stg = xpool.tile(list(dst.shape), F32, tag="wstage",
                                 bufs=3, name=stage_name)
                nc.sync.dma_start(out=stg[:], in_=src_ap)
                nc.vector.tensor_copy(dst[:], stg[:])

        for c in range(NCC):
            cs = slice(128 * c, 128 * (c + 1))
            load_rounded(wq_t[c], wqT[cs, :], f"sq{c}")
            load_rounded(wk_t[c], wkT[cs, :], f"sk{c}")
            load_rounded(wv_t[c], wvT[cs, :], f"sv{c}")
        for p in range(NPAIR):
            load_rounded(wp_t[p], wpT[128 * p:128 * (p + 1), :], f"sp{p}")
        for j in range(NJ):
            nc.vector.memset(vt[j][:], 1.0)

        for rep in range(reps):
            qTq = {}   # p -> current quarter's q tile
            yTq = {}   # (p, q) -> y tile (consumed by proj of that quarter)

            def qkv_block(tb, rep=rep):
                ts = slice(QW * tb, QW * (tb + 1))
                xt = []
                for c in range(NCC):
                    xc = xpool.tile([128, QW], D, tag="xt",
                                    name=f"x{rep}_{tb}_{c}")
                    if D is F32:
                        nc.sync.dma_start(out=xc[:],
                                          in_=xT[128 * c:128 * (c + 1), ts])
                    else:
                        stg = xpool.tile([128, QW], F32, tag="xstage", bufs=3,
                                         name=f"xs{rep}_{tb}_{c}")
                        nc.sync.dma_start(out=stg[:],
                                          in_=xT[128 * c:128 * (c + 1), ts])
                        nc.vector.tensor_copy(xc[:], stg[:])
                    xt.append(xc)
                for p in range(NPAIR):
                    ms = slice(128 * p, 128 * (p + 1))
                    qt = qpool.tile([128, QW], D, tag=f"qT{p}",
                                    name=f"qT{rep}_{tb}_{p}")
                    qTq[p] = qt
                    for w_t, b_t, dst in ((wq_t, bq_t, qt), (wk_t, bk_t, kTq[p][tb])):
                        ps = psum.tile([128, QW], F32, tag="mm", bufs=2,
                                       name=f"qk{rep}_{tb}_{p}")
                        for c in range(NCC):
                            nc.tensor.matmul(ps[:], w_t[c][:, ms], xt[c][:],
                                             start=(c == 0), stop=(c == NCC - 1))
                        nc.vector.tensor_scalar_add(dst[:], ps[:], b_t[:, p:p + 1])
                for s4 in range(R):
                    j = tb * R + s4
                    ps = psum.tile([128, MLOC], F32, tag="mm", bufs=2,
                                   name=f"vp{rep}_{j}")
                    for c in range(NCC):
                        nc.tensor.matmul(ps[:], xt[c][:, 128 * s4:128 * (s4 + 1)],
                                         wv_t[c][:], start=(c == 0),
                                         stop=(c == NCC - 1))
                    nc.vector.tensor_copy(
                        vt[j].rearrange("p (h d) -> p h d", d=65)[:, :, 0:64],
                        ps[:])

            def att_block(q, rep=rep):
                qs = slice(QW * q, QW * (q + 1))
                for p in range(NPAIR):
                    y = [psum.tile([128, QW], F32, tag=f"y{h}", bufs=1,
                                   name=f"y{rep}_{p}_{q}_{h}") for h in (0, 1)]
                    nj_q = R * (q + 1)
                    for j in range(nj_q):
                        o = max(0, 128 * j - QW * q)
                        w = QW - o
                        for h in (0, 1):
                            hs = slice(64 * h, 64 * (h + 1))
                            s_ps = psum.tile([128, w], F32, tag="st", bufs=2,
                                             name=f"s{rep}_{p}_{q}_{j}_{h}")
                            nc.tensor.matmul(
                                s_ps[:], kTq[p][j // R][hs, 128 * (j % R):128 * (j % R) + 128],
                                qTq[p][hs, o:QW],
                                start=True, stop=True)
                            pt = ppool.tile([128, w], D, tag="pt", bufs=4,
                                            name=f"pt{rep}_{p}_{q}_{j}_{h}")
                            nc.scalar.activation(pt[:], s_ps[:], EXP,
                                                 bias=rl_t[:, j:j + 1],
                                                 scale=0.125)
                            if j >= R * q:
                                nc.vector.tensor_mul(pt[:, 0:128],
                                                     pt[:, 0:128], tri_t[:])
                            lh = 2 * p + h
                            nc.tensor.matmul(
                                y[h][0:65, o:o + w],
                                vt[j][:, 65 * lh:65 * lh + 65], pt[:],
                                start=(j == 0), stop=(j == nj_q - 1))
                    yt = ypool.tile([128, QW], D, tag=f"yT{p}",
                                    name=f"yT{rep}_{p}_{q}")
                    yTq[(p, q)] = yt
                    for h in (0, 1):
                        den = spool.tile([128, QW], F32, tag="den", bufs=1,
                                         name=f"den{rep}_{p}_{q}_{h}")
                        nc.vector.tensor_copy(den[64:65, :], y[h][64:65, :])
                        bcp = psum.tile([64, QW], F32, tag="bc", bufs=1,
                                        name=f"bc{rep}_{p}_{q}_{h}")
                        nc.tensor.matmul(bcp[0:64, :], ones_t[64:65, 0:64],
                                         den[64:65, :], start=True, stop=True)
                        bcs = spool.tile([64, QW], F32, tag="bcs", bufs=1,
                                         name=f"bcs{rep}_{p}_{q}_{h}")
                        nc.vector.tensor_copy(bcs[:], bcp[0:64, :])
                        rbc = spool.tile([64, QW], F32, tag="rbc", bufs=1,
                                         name=f"rbc{rep}_{p}_{q}_{h}")
                        scr = spool.tile([64, QW], F32, tag="scrtmp", bufs=1,
                                         name=f"scr{rep}_{p}_{q}_{h}")
                        nc.vector.reciprocal_approx_accurate(
                            out=rbc[:], in_=bcs[:], scratch=scr[:])
                        if h == 0:
                            nc.vector.tensor_mul(yt[0:64, :], y[h][0:64, :], rbc[:])
                        else:
                            tmp = spool.tile([64, QW], D, tag="tmp", bufs=1,
                                             name=f"tmp{rep}_{p}_{q}")
                            nc.vector.tensor_mul(tmp[:], y[h][0:64, :], rbc[:])
                            nc.sync.dma_start(out=yt[64:128, :], in_=tmp[:])

            def proj_block(q, rep=rep):
                for ih in range(C // 512):
                    for l4 in range(R):
                        tb16 = q * R + l4
                        op = psum.tile([128, 512], F32, tag="mm", bufs=2,
                                       name=f"op{rep}_{ih}_{tb16}")
                        for p in range(NPAIR):
                            nc.tensor.matmul(
                                op[:], yTq[(p, q)][:, 128 * l4:128 * (l4 + 1)],
                                wp_t[p][:, 512 * ih:512 * (ih + 1)],
                                start=(p == 0), stop=(p == NPAIR - 1))
                        ob = opool.tile([128, 512], F32, tag="ob",
                                        name=f"ob{rep}_{ih}_{tb16}")
                        nc.vector.tensor_copy(ob[:], op[:])
                        nc.sync.dma_start(
                            out=out_d[128 * tb16:128 * (tb16 + 1),
                                      512 * ih:512 * (ih + 1)],
                            in_=ob[:])

            qkv_block(0)
            att_block(0)
            qkv_block(1)
            att_block(1)
            proj_block(0)
            qkv_block(2)
            att_block(2)
            proj_block(1)
            qkv_block(3)
            att_block(3)
            proj_block(2)
            proj_block(3)

    return nc

